# revision 5
# baseline (speedup 1.0000x reference)
"""BitGQA attention kernel for 8 trn2 NeuronCores.

Sharding: 8 cores = 2 batch groups x 4 tensor-parallel groups.
Core c handles batch b = c//4 and head-group g = c%4 (8 q heads, 2 kv heads,
512-wide slices of the q/o projections, 128-wide slices of k/v).

Key design points:
  - weights are ternary-quantized ON HOST and shipped as bf16 [D, width]
    (exact: ternary in {-1,0,1} and integer activations are exact in bf16
    matmuls); no on-device weight pass, no |w| AllReduce.
  - x is absmax-quantized per token (ACT Square+accum for RMS, DVE abs-max),
    rounded via the fp32 magic-constant trick, and transposed to [d, t]
    through the PE (identity matmul into PSUM, 4 token tiles per eviction)
    instead of DMA-crossbar transposes.
  - RoPE rotate_half runs as a PE permutation matmul (host-shipped [128,128]
    permutation), sign baked into the sin table; no partition-shift DMAs.
  - causal attention per head: scoresT = kT.T @ qT with exp batched over
    full k-tile pairs ([128,1024] ACT ops); only the 4 diagonal k-tiles run
    per-tile with a triangular mask; attnV uses a [v|1] augmented stationary
    operand so the softmax denominator falls out of the same matmul.
  - ONE collective: ao is normalized by the softmax denominators (bf16) and
    AllGathered raw; every core then computes the global RMS/absmax stats
    (PE ones-matmul for cross-partition sums, DVE max-tree + PE transpose
    for absmax), quantizes in SBUF, and feeds its o-projection slice
    straight from SBUF - no quantized-activation round trip through DRAM.

The final output is assembled on host from the 8 [2048, 512] slices.
"""

import contextlib

import numpy as np

import concourse.bass as bass
import concourse.bacc as bacc
import concourse.mybir as mybir
import concourse.tile as tile
from concourse import bass_utils

F32 = mybir.dt.float32
BF16 = mybir.dt.bfloat16
AF = mybir.ActivationFunctionType
ALU = mybir.AluOpType

MAGIC = float(1.5 * 2.0**23)  # fp32 round-to-nearest-even magic constant
EPS_NORM = 1e-6
EPS_Q = 1e-5

N_CORES = 8
D = 2048
H_TOTAL, KV_TOTAL, HD = 32, 8, 64
G = 4  # tensor-parallel groups
NH = H_TOTAL // G          # 8 local q heads
NKV = KV_TOTAL // G        # 2 local kv heads
QO = NH * HD               # 512 local q dims
KO = NKV * HD              # 128 local kv dims
ND = D // 128              # 16 d-tiles


def build_program(T=2048, has_g=False, n_cores=N_CORES,
                  emulate_collectives=False):
    NT = T // 128   # token tiles
    NJ = T // 512   # 512-wide token columns
    NO = QO // 128  # q/o o-tiles (4)
    rg = ([[0, 1, 2, 3], [4, 5, 6, 7]] if n_cores == N_CORES else
          [[c] for c in range(n_cores)])

    nc = bacc.Bacc("TRN2", target_bir_lowering=False, debug=False,
                   num_devices=n_cores)

    # ---- per-core DRAM I/O ----
    x_d = nc.dram_tensor("x", [T, D], F32, kind="ExternalInput")
    wqt_d = nc.dram_tensor("wqt_b", [D, QO], BF16, kind="ExternalInput")
    wkt_d = nc.dram_tensor("wkt_b", [D, KO], BF16, kind="ExternalInput")
    wvt_d = nc.dram_tensor("wvt_b", [D, KO], BF16, kind="ExternalInput")
    wot_d = nc.dram_tensor("wot_b", [D, QO], BF16, kind="ExternalInput")
    wsc_d = nc.dram_tensor("wsc", [1, 4], F32, kind="ExternalInput")
    cost2_d = nc.dram_tensor("cost2", [128, T], BF16, kind="ExternalInput")
    sint2s_d = nc.dram_tensor("sint2s", [128, T], BF16, kind="ExternalInput")
    go_d = nc.dram_tensor("go_r", [1, QO], F32, kind="ExternalInput")
    perm_d = nc.dram_tensor("perm", [128, 128], BF16, kind="ExternalInput")
    if has_g:
        g_d = nc.dram_tensor("g_r", [1, D], F32, kind="ExternalInput")
    out_d = nc.dram_tensor("out", [T, QO], F32, kind="ExternalOutput")

    with tile.TileContext(nc) as tc, contextlib.ExitStack() as stack:
        # ---------------- long-lived pools (strict stack order) --------------
        singles = stack.enter_context(tc.tile_pool(name="singles", bufs=1))
        cols = stack.enter_context(tc.tile_pool(name="cols", bufs=1))
        dram = stack.enter_context(tc.tile_pool(name="dram", bufs=1,
                                                space="DRAM"))

        # constants
        zero_col = singles.tile([128, 1], F32)
        nc.vector.memset(zero_col, 0.0)
        magic_col = singles.tile([128, 1], F32)
        nc.vector.memset(magic_col, MAGIC)
        epsn_col = singles.tile([128, 1], F32)
        nc.vector.memset(epsn_col, EPS_NORM)

        identity = singles.tile([128, 128], BF16)
        nc.gpsimd.memset(identity, 1.0)
        nc.gpsimd.affine_select(out=identity, in_=identity, compare_op=ALU.is_ge,
                                fill=0.0, base=0, pattern=[[-1, 128]],
                                channel_multiplier=1)
        nc.gpsimd.affine_select(out=identity, in_=identity, compare_op=ALU.is_ge,
                                fill=0.0, base=0, pattern=[[1, 128]],
                                channel_multiplier=-1)

        # causal mask for diagonal 128x128 blocks of scoresT[k, t]:
        # keep 1.0 where t >= k i.e. (free - partition) >= 0
        trimask = singles.tile([128, 128], BF16)
        nc.gpsimd.memset(trimask, 1.0)
        nc.gpsimd.affine_select(out=trimask, in_=trimask, compare_op=ALU.is_ge,
                                fill=0.0, base=0, pattern=[[1, 128]],
                                channel_multiplier=-1)

        # go as per-head columns [64, NH]
        go_cols = singles.tile([64, NH], F32)
        nc.sync.dma_start(out=go_cols,
                          in_=go_d[0:1, :].rearrange("1 (h p) -> p h", p=64))

        # rotate-half permutation (signed) for RoPE via PE
        perm = singles.tile([128, 128], BF16)
        nc.sync.dma_start(out=perm, in_=perm_d[:, :])

        # weight dequant scales broadcast to [128, 4]
        ws_cols = singles.tile([128, 4], F32)
        nc.sync.dma_start(out=ws_cols, in_=wsc_d[0:1, :].to_broadcast((128, 4)))

        if has_g:
            g_bcast = singles.tile([128, D], F32)
            nc.sync.dma_start(out=g_bcast, in_=g_d[0:1, :].to_broadcast((128, D)))

        # ====== lifetime pools, opened in reverse-close order ================
        es_wo = contextlib.ExitStack()
        wo_pool = es_wo.enter_context(tc.tile_pool(name="wop", bufs=1))
        es_ao = contextlib.ExitStack()
        ao_pool = es_ao.enter_context(tc.tile_pool(name="aop", bufs=1))
        es_qkv = contextlib.ExitStack()
        qkv_pool = es_qkv.enter_context(tc.tile_pool(name="qkv", bufs=1))
        es_proj = contextlib.ExitStack()
        proj_pool = es_proj.enter_context(tc.tile_pool(name="proj", bufs=1))

        # ---------------- phase W: load pre-quantized ternary weights --------
        wo_bf = [wo_pool.tile([128, QO], BF16, name=f"wo_bf{r}")
                 for r in range(ND)]
        wq_bf = [proj_pool.tile([128, QO], BF16, name=f"wq_bf{r}")
                 for r in range(ND)]
        wk_bf = [proj_pool.tile([128, KO], BF16, name=f"wk_bf{r}")
                 for r in range(ND)]
        wv_bf = [proj_pool.tile([128, KO], BF16, name=f"wv_bf{r}")
                 for r in range(ND)]
        for r in range(ND):
            rs = slice(r * 128, (r + 1) * 128)
            nc.gpsimd.dma_start(out=wq_bf[r], in_=wqt_d[rs, :])
            nc.gpsimd.dma_start(out=wk_bf[r], in_=wkt_d[rs, :])
            nc.gpsimd.dma_start(out=wv_bf[r], in_=wvt_d[rs, :])

        cost2 = proj_pool.tile([128, T], BF16)
        nc.gpsimd.dma_start(out=cost2, in_=cost2_d[:, :])
        sint2s = proj_pool.tile([128, T], BF16)
        nc.gpsimd.dma_start(out=sint2s, in_=sint2s_d[:, :])

        # attention operands (qkv lifetime)
        qT = [qkv_pool.tile([128, T], BF16, name=f"qT{a}") for a in range(NO)]
        kT = qkv_pool.tile([128, T], BF16)
        v1 = [[qkv_pool.tile([128, HD + 1], BF16, name=f"v1_{kv}_{r}")
               for r in range(NT)] for kv in range(NKV)]
        for kv in range(NKV):
            for r in range(NT):
                nc.vector.memset(v1[kv][r][:, HD:HD + 1], 1.0)

        # x-quant stat columns
        ss_col = cols.tile([128, NT], F32)
        amax_col = cols.tile([128, NT], F32)
        rsq_col = cols.tile([128, NT], F32)
        xsc_col = cols.tile([128, NT], F32)
        s_col = cols.tile([128, NT], F32)
        scr1_col = cols.tile([128, NT], F32)
        fv_col = cols.tile([128, NT], F32)
        xsc_d = dram.tile([1, T], F32)

        def rope(dst_tiles, n_tiles, j, pool, pspool):
            # q_rope = q*cos + rotate_half(q)*sin; rotate_half is a signed
            # partition permutation done on the PE (perm matmul)
            jc = slice(j * 512, (j + 1) * 512)
            for a in range(n_tiles):
                t = dst_tiles[a]
                ps_sh = pspool.tile([128, 512], F32, tag="rope", bufs=1)
                nc.tensor.matmul(ps_sh, perm, t[:, jc], start=True, stop=True)
                tmp = pool.tile([128, 512], BF16, tag="ropetmp")
                nc.vector.tensor_tensor(out=tmp, in0=ps_sh, in1=sint2s[:, jc],
                                        op=ALU.mult)
                sh = pool.tile([128, 512], BF16, tag="ropesh")
                nc.vector.tensor_tensor(out=sh, in0=t[:, jc],
                                        in1=cost2[:, jc], op=ALU.mult)
                nc.vector.tensor_tensor(out=t[:, jc], in0=sh, in1=tmp,
                                        op=ALU.add)

        # -------- phases X+P merged: stream token columns of 512 -------------
        with tc.tile_pool(name="xpool", bufs=2) as xpool, \
             tc.tile_pool(name="xscr", bufs=1) as xscr, \
             tc.tile_pool(name="xqTc", bufs=2) as xqTc_pool, \
             tc.tile_pool(name="fqfp", bufs=1) as fqfp, \
             tc.tile_pool(name="psq", bufs=1, space="PSUM") as psq, \
             tc.tile_pool(name="psk", bufs=1, space="PSUM") as psk, \
             tc.tile_pool(name="psv", bufs=1, space="PSUM") as psv, \
             tc.tile_pool(name="psx", bufs=2, space="PSUM") as psx, \
             tc.tile_pool(name="ropep", bufs=1) as rope_pool:
            for j in range(NJ):
                jc = slice(j * 512, (j + 1) * 512)
                xqTc = [xqTc_pool.tile([128, 512], BF16, tag=f"xqT{r}",
                                       name=f"xqTc{r}") for r in range(ND)]
                # ---- X: quantize 4 token tiles of this column ----
                cj = slice(4 * j, 4 * j + 4)
                xts = []
                for s4 in range(4):
                    i = 4 * j + s4
                    ci = slice(i, i + 1)
                    xt = xpool.tile([128, D], F32, tag=f"xt{s4}",
                                    name=f"xt{s4}", bufs=1)
                    nc.sync.dma_start(out=xt,
                                      in_=x_d[i * 128:(i + 1) * 128, :])
                    if has_g:
                        xg = xpool.tile([128, D], F32, tag=f"xg{s4}",
                                        name=f"xg{s4}", bufs=1)
                        nc.vector.tensor_tensor(out=xg, in0=xt, in1=g_bcast,
                                                op=ALU.mult)
                        src = xg
                    else:
                        src = xt
                    xts.append(src)
                    # discard target for Square reuses the xtmp slot bytes
                    sq_scr = xscr.tile([128, D], F32, tag="xtmp")
                    nc.scalar.activation(sq_scr.bitcast(BF16)[:, 0:D], xt,
                                         AF.Square, bias=zero_col,
                                         scale=1.0, accum_out=ss_col[:, ci])
                    nc.vector.tensor_reduce(out=amax_col[:, ci], in_=src,
                                            axis=mybir.AxisListType.X,
                                            op=ALU.max,
                                            apply_absolute_value=True)
                # batched per-column stat math on [128, 4] slices
                nc.scalar.activation(scr1_col[:, cj], ss_col[:, cj],
                                     AF.Sqrt, bias=epsn_col, scale=1.0 / D)
                nc.vector.reciprocal(rsq_col[:, cj], scr1_col[:, cj])
                nc.vector.tensor_tensor(out=xsc_col[:, cj],
                                        in0=amax_col[:, cj],
                                        in1=rsq_col[:, cj], op=ALU.mult)
                nc.vector.tensor_scalar_max(xsc_col[:, cj], xsc_col[:, cj],
                                            EPS_Q)
                nc.vector.reciprocal(scr1_col[:, cj], xsc_col[:, cj])
                nc.vector.tensor_tensor(out=s_col[:, cj], in0=rsq_col[:, cj],
                                        in1=scr1_col[:, cj], op=ALU.mult)
                nc.vector.tensor_scalar_mul(s_col[:, cj], s_col[:, cj], 127.0)
                xqs = []
                for s4 in range(4):
                    i = 4 * j + s4
                    tmp = xscr.tile([128, D], F32, tag="xtmp")
                    nc.scalar.activation(tmp, xts[s4], AF.Identity,
                                         bias=magic_col,
                                         scale=s_col[:, i:i + 1])
                    xq = xts[s4].bitcast(BF16)[:, 0:D]
                    nc.vector.tensor_scalar_sub(xq, tmp, MAGIC)
                    xqs.append(xq)
                # PE transpose: 4 token tiles land in one [128, 512] PSUM
                # tile (column-offset writes), single eviction per d-tile
                for r in range(ND):
                    pst = psx.tile([128, 512], BF16, tag="pst")
                    for s4 in range(4):
                        nc.tensor.transpose(pst[:, s4 * 128:(s4 + 1) * 128],
                                            xqs[s4][:, r * 128:(r + 1) * 128],
                                            identity)
                    if r % 2 == 0:
                        nc.vector.tensor_copy(out=xqTc[r], in_=pst)
                    else:
                        nc.scalar.copy(out=xqTc[r], in_=pst)

                # ---- dequant factor chunks for this column ----
                nc.sync.dma_start(
                    out=xsc_d[0:1, jc].rearrange("1 (i p) -> p i", p=128),
                    in_=xsc_col[:, 4 * j:4 * j + 4])
                fq_f = fqfp.tile([128, 512], F32, tag="fqf")
                nc.sync.dma_start(out=fq_f,
                                  in_=xsc_d[0:1, jc].to_broadcast((128, 512)))
                fqc = fqfp.tile([128, 512], BF16, tag="fqc")
                fkc = fqfp.tile([128, 512], BF16, tag="fkc")
                nc.vector.tensor_scalar(fqc, fq_f, ws_cols[:, 0:1],
                                        1.0 / 127.0, op0=ALU.mult, op1=ALU.mult)
                nc.vector.tensor_scalar(fkc, fq_f, ws_cols[:, 1:2],
                                        1.0 / (127.0 * float(np.sqrt(HD))),
                                        op0=ALU.mult, op1=ALU.mult)
                nc.vector.tensor_scalar(fv_col[:, 4 * j:4 * j + 4],
                                        xsc_col[:, 4 * j:4 * j + 4],
                                        ws_cols[:, 2:3], 1.0 / 127.0,
                                        op0=ALU.mult, op1=ALU.mult)

                # ---- P: projections for this column ----
                ps_k = psk.tile([128, 512], F32)
                ps_v = psv.tile([128, 512], F32)
                for r in range(ND):
                    ch = xqTc[r]
                    st = dict(start=(r == 0), stop=(r == ND - 1))
                    nc.tensor.matmul(ps_k, wk_bf[r], ch, **st)
                for s in range(4):
                    for r in range(ND):
                        nc.tensor.matmul(ps_v[:, s * 128:(s + 1) * 128],
                                         xqTc[r][:, s * 128:(s + 1) * 128],
                                         wv_bf[r], start=(r == 0),
                                         stop=(r == ND - 1))
                nc.vector.tensor_tensor(out=kT[:, jc], in0=ps_k, in1=fkc,
                                        op=ALU.mult)
                for ah in range(2):
                    ps_q = [psq.tile([128, 512], F32, tag=f"q{a2}",
                                     name=f"ps_q{ah}_{a2}")
                            for a2 in range(2)]
                    for r in range(ND):
                        ch = xqTc[r]
                        st = dict(start=(r == 0), stop=(r == ND - 1))
                        for a2 in range(2):
                            a = 2 * ah + a2
                            nc.tensor.matmul(ps_q[a2],
                                             wq_bf[r][:, a * 128:(a + 1) * 128],
                                             ch, **st)
                    for a2 in range(2):
                        a = 2 * ah + a2
                        nc.vector.tensor_tensor(out=qT[a][:, jc],
                                                in0=ps_q[a2],
                                                in1=fqc, op=ALU.mult)
                for s in range(4):
                    kt_i = 4 * j + s
                    for kv in range(NKV):
                        nc.vector.tensor_scalar_mul(
                            v1[kv][kt_i][:, 0:HD],
                            ps_v[:, s * 128 + kv * HD:s * 128 + (kv + 1) * HD],
                            fv_col[:, kt_i:kt_i + 1])
                rope(qT, NO, j, rope_pool, psx)
                rope([kT], 1, j, rope_pool, psx)

        # wo is first needed in phase O: load it outside the startup burst
        for r in range(ND):
            nc.sync.dma_start(out=wo_bf[r],
                              in_=wot_d[r * 128:(r + 1) * 128, :])

        es_proj.close()  # frees weights, rope tables, xqT column tiles

        # kT with kv halves swapped so every q head finds its kv head at its
        # own base partition (matmul requires equal base partitions)
        kT2 = qkv_pool.tile([128, T], BF16)
        nc.vector.tensor_copy(out=kT2[0:64, :], in_=kT[64:128, :])
        nc.vector.tensor_copy(out=kT2[64:128, :], in_=kT[0:64, :])

        ao = [ao_pool.tile([128, T], BF16, name=f"ao{a}") for a in range(NO)]
        rsums_d = dram.tile([NH, T], F32)
        xqo_in = dram.tile([QO, T], BF16)
        xqo_out = dram.tile([G * QO, T], BF16)

        # ---------------- phase A: attention --------------------------------
        # Full (below-diagonal) k-tiles are processed in pairs: two score
        # matmuls into one [128, 1024] 2-bank PSUM tile, one exp, two attnV
        # matmuls. The 4 diagonal k-tiles keep the per-tile c0/trimask path.
        with tc.tile_pool(name="psa", bufs=2, space="PSUM") as psa, \
             tc.tile_pool(name="psd", bufs=2, space="PSUM") as psd, \
             tc.tile_pool(name="pso", bufs=2, space="PSUM") as pso, \
             tc.tile_pool(name="ptp", bufs=4) as ptp, \
             tc.tile_pool(name="sump", bufs=2) as sump, \
             tc.tile_pool(name="facp", bufs=2) as facp:
            for h in range(NH):
                kv = h // (NH // NKV)
                a_t, pr = h // 2, (h % 2) * 64
                qh = qT[a_t][pr:pr + 64, :]
                ksrc = kT if kv * HD == pr else kT2
                kh = ksrc[pr:pr + 64, :]
                sumstage = sump.tile([1, T], F32, tag="sumstage")
                for j in range(NJ):
                    nk = 4 * (j + 1)
                    ps_o = pso.tile([128, 512], F32, tag="o")
                    # full k-tile pairs below the diagonal block row
                    for p2 in range(2 * j):
                        r0 = 2 * p2
                        ps_s = psa.tile([128, 1024], F32, tag="s")
                        for u in (0, 1):
                            r = r0 + u
                            nc.tensor.matmul(
                                ps_s[:, u * 512:(u + 1) * 512],
                                kh[:, r * 128:(r + 1) * 128],
                                qh[:, jc_sl(j)], start=True, stop=True)
                        pt = ptp.tile([128, 1024], BF16, tag="pt")
                        nc.scalar.activation(pt, ps_s, AF.Exp, bias=zero_col,
                                             scale=1.0)
                        for u in (0, 1):
                            r = r0 + u
                            nc.tensor.matmul(ps_o[0:HD + 1, :],
                                             v1[kv][r],
                                             pt[:, u * 512:(u + 1) * 512],
                                             start=(r == 0), stop=False,
                                             skip_group_check=True)
                    # 4 diagonal k-tiles (r = 4j..4j+3)
                    for phi in range(4):
                        r = 4 * j + phi
                        c0 = 128 * phi
                        ps_sd = psd.tile([128, 512], F32, tag="sd")
                        nc.tensor.matmul(
                            ps_sd[:, c0:512], kh[:, r * 128:(r + 1) * 128],
                            qh[:, j * 512 + c0:(j + 1) * 512],
                            start=True, stop=True)
                        pt = ptp.tile([128, 512], BF16, tag="ptd")
                        nc.scalar.activation(pt[:, c0:512], ps_sd[:, c0:512],
                                             AF.Exp, bias=zero_col, scale=1.0)
                        nc.vector.tensor_tensor(
                            out=pt[:, c0:c0 + 128], in0=pt[:, c0:c0 + 128],
                            in1=trimask, op=ALU.mult)
                        # columns < c0 are fully masked: skip them instead of
                        # zero-filling (they were started by earlier k-tiles)
                        nc.tensor.matmul(ps_o[0:HD + 1, c0:512],
                                         v1[kv][r], pt[:, c0:512],
                                         start=(r == 0), stop=(r == nk - 1),
                                         skip_group_check=True)
                    jc = slice(j * 512, (j + 1) * 512)
                    nc.vector.tensor_copy(out=sumstage[0:1, jc],
                                          in_=ps_o[HD:HD + 1, :])
                    nc.vector.tensor_scalar_mul(ao[a_t][pr:pr + 64, jc],
                                                ps_o[0:HD, :],
                                                go_cols[:, h:h + 1])
                rstage = sump.tile([1, T], F32, tag="rstage")
                nc.vector.reciprocal(rstage, sumstage)
                nc.sync.dma_start(out=rsums_d[h:h + 1, :], in_=rstage)
                if h % 2 == 1:
                    # normalize the finished head pair while later heads
                    # still run: ao_n = ao / softmax-denominator (bf16)
                    a = h // 2
                    rsb = facp.tile([128, T], BF16, tag="rsb")
                    nc.gpsimd.dma_start(
                        out=rsb[0:64, :],
                        in_=rsums_d[2 * a:2 * a + 1, :].to_broadcast((64, T)))
                    nc.gpsimd.dma_start(
                        out=rsb[64:128, :],
                        in_=rsums_d[2 * a + 1:2 * a + 2, :]
                        .to_broadcast((64, T)))
                    aon = facp.tile([128, T], BF16, tag="aon")
                    nc.vector.tensor_tensor(out=aon, in0=ao[a], in1=rsb,
                                            op=ALU.mult)
                    nc.sync.dma_start(out=xqo_in[a * 128:(a + 1) * 128, :],
                                      in_=aon)

        es_qkv.close()  # frees qT, kT, kT2, v1

        es_ao.close()  # frees ao, sums, rsums
        if emulate_collectives:
            for p in range(G):
                nc.sync.dma_start(out=xqo_out[p * QO:(p + 1) * QO, :],
                                  in_=xqo_in[:])
        else:
            nc.gpsimd.collective_compute("AllGather", ALU.bypass,
                                         replica_groups=rg,
                                         ins=[xqo_in.opt()],
                                         outs=[xqo_out.opt()])

        # -------- phase OS: global stats + quantize + o-projection -----------
        ones_bf = singles.tile([128, 1], BF16)
        nc.vector.memset(ones_bf, 1.0)
        ss_row_d = dram.tile([1, T], F32)
        so_d = dram.tile([1, T], F32)
        amax2_col = cols.tile([128, NT], BF16)
        ss_colg = cols.tile([128, NT], F32)
        m1c = cols.tile([128, NT], F32)
        r1c = cols.tile([128, NT], F32)
        t2c = cols.tile([128, NT], F32)
        xsc_o = cols.tile([128, NT], F32)
        rsq_o = cols.tile([128, NT], F32)
        so_col = cols.tile([128, NT], F32)
        fo_col = cols.tile([128, NT], F32)

        with tc.tile_pool(name="xqgp", bufs=1) as xqgp, \
             tc.tile_pool(name="sqp", bufs=2) as sqp, \
             tc.tile_pool(name="amxp", bufs=1) as amxp, \
             tc.tile_pool(name="ssrp", bufs=1) as ssrp, \
             tc.tile_pool(name="psst", bufs=1, space="PSUM") as psst, \
             tc.tile_pool(name="pstr", bufs=2, space="PSUM") as pstr, \
             tc.tile_pool(name="sobp", bufs=1) as sobp, \
             tc.tile_pool(name="qtmp", bufs=2) as qtmp, \
             tc.tile_pool(name="psf", bufs=2, space="PSUM") as psf, \
             tc.tile_pool(name="outp", bufs=2) as outp:
            xq_g = [xqgp.tile([128, T], BF16, name=f"xq_g{r}")
                    for r in range(ND)]
            amx = amxp.tile([128, T], BF16)
            ps_ss = psst.tile([1, T], F32)
            for r in range(ND):
                eng = nc.sync if r % 2 == 0 else nc.gpsimd
                eng.dma_start(out=xq_g[r],
                              in_=xqo_out[r * 128:(r + 1) * 128, :])
                sq = sqp.tile([128, T], BF16, tag="sq")
                nc.scalar.activation(sq, xq_g[r], AF.Square, bias=zero_col,
                                     scale=1.0)
                if r == 0:
                    nc.vector.tensor_copy(out=amx, in_=sq)
                else:
                    nc.vector.tensor_tensor(out=amx, in0=amx, in1=sq,
                                            op=ALU.max)
                for c in range(NJ):
                    cs = slice(c * 512, (c + 1) * 512)
                    nc.tensor.matmul(ps_ss[0:1, cs], ones_bf, sq[:, cs],
                                     start=(r == 0), stop=(r == ND - 1))
            # per-token amax^2 -> column layout via PE transpose + DVE reduce
            for i in range(NT):
                ptr = pstr.tile([128, 128], BF16, tag="ptr")
                nc.tensor.transpose(ptr, amx[:, i * 128:(i + 1) * 128],
                                    identity)
                nc.vector.tensor_reduce(out=amax2_col[:, i:i + 1], in_=ptr,
                                        axis=mybir.AxisListType.X, op=ALU.max)
            # ss rows -> DRAM -> column layout
            ss_row = ssrp.tile([1, T], F32)
            nc.scalar.copy(out=ss_row, in_=ps_ss)
            nc.sync.dma_start(out=ss_row_d[0:1, :], in_=ss_row)
            nc.sync.dma_start(
                out=ss_colg,
                in_=ss_row_d[0:1, :].rearrange("1 (i p) -> p i", p=128))

            # stat math: xsc = sqrt(amax^2 / (ms + eps)), so = 127*rsqrt/xsc
            nc.scalar.activation(m1c, ss_colg, AF.Identity, bias=epsn_col,
                                 scale=1.0 / (H_TOTAL * HD))
            nc.vector.reciprocal(r1c, m1c)
            nc.vector.tensor_tensor(out=t2c, in0=amax2_col, in1=r1c,
                                    op=ALU.mult)
            nc.scalar.activation(xsc_o, t2c, AF.Sqrt, bias=zero_col, scale=1.0)
            nc.vector.tensor_scalar_max(xsc_o, xsc_o, EPS_Q)
            nc.scalar.activation(rsq_o, r1c, AF.Sqrt, bias=zero_col, scale=1.0)
            nc.vector.reciprocal(so_col, xsc_o)
            nc.vector.tensor_tensor(out=so_col, in0=so_col, in1=rsq_o,
                                    op=ALU.mult)
            nc.vector.tensor_scalar_mul(so_col, so_col, 127.0)
            nc.vector.tensor_scalar(fo_col, xsc_o, ws_cols[:, 3:4], 1.0 / 127.0,
                                    op0=ALU.mult, op1=ALU.mult)
            nc.sync.dma_start(
                out=so_d[0:1, :].rearrange("1 (i p) -> p i", p=128),
                in_=so_col)
            so_b = sobp.tile([128, T], F32)
            nc.gpsimd.dma_start(out=so_b, in_=so_d.to_broadcast((128, T)))

            # quantize in place per 512-token chunk, o-project that chunk
            for c in range(NJ):
                cs = slice(c * 512, (c + 1) * 512)
                for r in range(ND):
                    tmp = qtmp.tile([128, 512], F32, tag="qtmp")
                    nc.vector.tensor_tensor(out=tmp, in0=xq_g[r][:, cs],
                                            in1=so_b[:, cs], op=ALU.mult)
                    nc.vector.tensor_scalar(xq_g[r][:, cs], tmp, MAGIC, MAGIC,
                                            op0=ALU.add, op1=ALU.subtract)
                for i in range(4 * c, 4 * c + 4):
                    ps_f = psf.tile([128, 512], F32, tag="f")
                    for r in range(ND):
                        nc.tensor.matmul(ps_f,
                                         xq_g[r][:, i * 128:(i + 1) * 128],
                                         wo_bf[r], start=(r == 0),
                                         stop=(r == ND - 1))
                    out_t = outp.tile([128, QO], F32, tag="out")
                    nc.scalar.activation(out_t, ps_f, AF.Copy, bias=0.0,
                                         scale=fo_col[:, i:i + 1])
                    nc.sync.dma_start(out=out_d[i * 128:(i + 1) * 128, :],
                                      in_=out_t)
        es_wo.close()

    nc.compile()
    return nc


def jc_sl(j):
    return slice(j * 512, (j + 1) * 512)


# ---------------------------------------------------------------------------
# host wrapper
# ---------------------------------------------------------------------------
_CACHE = {}


def _get_program(T, has_g):
    key = (T, has_g)
    if key not in _CACHE:
        _CACHE[key] = build_program(T=T, has_g=has_g)
    return _CACHE[key]


def _quant_w(w):
    """Host-side ternary quantization: returns (wT ternary bf16 [D, width],
    scale fp32)."""
    import ml_dtypes
    ws = max(np.mean(np.abs(w), dtype=np.float32), np.float32(EPS_Q))
    wq = np.clip(np.rint(w.T / ws), -1.0, 1.0)
    return np.ascontiguousarray(wq).astype(ml_dtypes.bfloat16), np.float32(ws)


def make_in_maps(x, cos, sin, wq, wk, wv, wo, gq, gk, gv, go, T):
    import ml_dtypes
    cosT = np.ascontiguousarray(cos.T.astype(np.float32))      # [64, T]
    sinT = np.ascontiguousarray(sin.T.astype(np.float32))
    cost2 = np.concatenate([cosT, cosT], axis=0)               # [128, T]
    sint_signed = np.concatenate([-sinT[0:32], sinT[32:64]], axis=0)
    sint2s = np.concatenate([sint_signed, sint_signed], axis=0)
    cost2 = cost2.astype(ml_dtypes.bfloat16)
    sint2s = sint2s.astype(ml_dtypes.bfloat16)

    ones = np.ones((D,), np.float32)
    has_g = not (np.array_equal(gq, ones) and np.array_equal(gk, ones)
                 and np.array_equal(gv, ones))
    if has_g:
        assert np.array_equal(gq, gk) and np.array_equal(gk, gv), \
            "per-projection norm weights must match"

    P = np.zeros((128, 128), np.float32)
    for blk in (0, 64):
        for i in range(32):
            P[blk + i + 32, blk + i] = 1.0
            P[blk + i, blk + i + 32] = 1.0
    P = P.astype(ml_dtypes.bfloat16)

    wq_b, ws_q = _quant_w(wq)
    wk_b, ws_k = _quant_w(wk)
    wv_b, ws_v = _quant_w(wv)
    wo_b, ws_o = _quant_w(wo)
    wsc = np.array([[ws_q, ws_k, ws_v, ws_o]], np.float32)

    xs = [np.ascontiguousarray(x[b].astype(np.float32)) for b in range(2)]
    in_maps = []
    for c in range(N_CORES):
        b, g = c // G, c % G
        m = {
            "x": xs[b],
            "wqt_b": np.ascontiguousarray(wq_b[:, g * QO:(g + 1) * QO]),
            "wkt_b": np.ascontiguousarray(wk_b[:, g * KO:(g + 1) * KO]),
            "wvt_b": np.ascontiguousarray(wv_b[:, g * KO:(g + 1) * KO]),
            "wot_b": np.ascontiguousarray(wo_b[:, g * QO:(g + 1) * QO]),
            "wsc": wsc,
            "cost2": cost2,
            "sint2s": sint2s,
            "go_r": np.ascontiguousarray(go[g * QO:(g + 1) * QO][None, :]),
            "perm": P,
        }
        if has_g:
            m["g_r"] = np.ascontiguousarray(gq[None, :])
        in_maps.append(m)
    return in_maps, has_g


def kernel(x, cos, sin, wq, wk, wv, wo, gq, gk, gv, go):
    x = np.asarray(x, np.float32)
    T = x.shape[1]
    in_maps, has_g = make_in_maps(x, cos, sin, np.asarray(wq, np.float32),
                                  np.asarray(wk, np.float32),
                                  np.asarray(wv, np.float32),
                                  np.asarray(wo, np.float32),
                                  np.asarray(gq, np.float32),
                                  np.asarray(gk, np.float32),
                                  np.asarray(gv, np.float32),
                                  np.asarray(go, np.float32), T)
    nc = _get_program(T, has_g)
    res = bass_utils.run_bass_kernel_spmd(nc, in_maps,
                                          core_ids=list(range(N_CORES)))
    out = np.empty((2, T, D), np.float32)
    for c in range(N_CORES):
        b, g = c // G, c % G
        out[b][:, g * QO:(g + 1) * QO] = res.results[c]["out"]
    return out
